# revision 1
# baseline (speedup 1.0000x reference)
"""Jacobi 100-step solver on 8 trn2 cores via truncated DST-spectral transform.

x_{t+1} = mask * (0.25 * 4-neighbor-sum)  is linear:  x' = A x + x A  with
A = 0.25*D*tridiag*D (D zeroes the boundary). Left/right multiplies commute, so
100 steps diagonalize in the DST basis Q: after one explicit step (x1 has zero
boundary), x100 = Q (s^99 ⊙ (Q x1 Q)) Q with s = 0.5(cosθa+cosθb). |s|^99 is
negligible outside the lowest-K and highest-K mode corners (K=384 → rel err
~3e-5), so only two [384,384] spectral blocks are computed. Sharding: column
panels of 256 per core; one 1.2MB AllReduce of the spectral blocks.
"""

import sys
import types
import numpy as np

N = 2048
NC = 8
P = N // NC          # 256 panel columns per core
K = 384              # spectral corner size
KC = K // 128        # 3 chunks
RC = N // 128        # 16 row chunks
PW = P + 2           # panel width with 1-col halos


def _install_ntff_hook():
    if "antenv.axon_hooks" in sys.modules:
        return
    mod = types.ModuleType("antenv.axon_hooks")
    mod._hook = None
    mod.set_axon_ntff_profile_hook = lambda h: setattr(mod, "_hook", h)
    mod.get_axon_ntff_profile_hook = lambda: mod._hook
    sys.modules["antenv.axon_hooks"] = mod
    try:
        import antenv
        antenv.axon_hooks = mod
        from trn_agent_boot.trn_boot import _ntff_profile_via_ctypes
        h = _ntff_profile_via_ctypes("/opt/axon/libaxon_pjrt.so")
        if h is not None:
            mod.set_axon_ntff_profile_hook(h)
    except Exception:
        pass


def _host_constants():
    i = np.arange(N, dtype=np.float64)
    consts = {}
    for t, lo in (("lo", True), ("hi", False)):
        m = np.arange(1, K + 1, dtype=np.float64) if lo else np.arange(N - 1 - K, N - 1, dtype=np.float64)
        # exact-period-reduced DST matrix, symmetric: Q[i,m] = sqrt(2/2047) sin(pi*i*m/2047)
        red = np.outer(i, m) % (2 * (N - 1))
        Qc = np.sqrt(2.0 / (N - 1)) * np.sin(np.pi * red / (N - 1))   # [2048, K]
        lam = 0.5 * np.cos(np.pi * m / (N - 1))
        W99 = (lam[:, None] + lam[None, :]) ** 99                     # [K, K]
        consts[f"qc_{t}"] = Qc.astype(np.float32)
        consts[f"w99_{t}"] = W99.astype(np.float32)
    smid = np.zeros((128, 128), np.float32)
    for d in range(127):
        smid[d, d + 1] = 1.0
        smid[d + 1, d] = 1.0
    sup = np.zeros((128, 128), np.float32); sup[127, 0] = 1.0
    sdn = np.zeros((128, 128), np.float32); sdn[0, 127] = 1.0
    consts["smid"], consts["sup"], consts["sdn"] = smid, sup, sdn
    consts["ident"] = np.eye(128, dtype=np.float32)
    return consts


_NC_CACHE = {}


def _build():
    if "nc" in _NC_CACHE:
        return _NC_CACHE["nc"]
    import concourse.bacc as bacc
    import concourse.tile as tile
    import concourse.mybir as mybir

    DT = mybir.dt.float32
    nc = bacc.Bacc("TRN2", target_bir_lowering=False, debug=False, num_devices=NC)

    xin = nc.dram_tensor("X", [N, PW], DT, kind="ExternalInput")
    yin = nc.dram_tensor("Y", [N, PW], DT, kind="ExternalInput")
    qc_d = {t: nc.dram_tensor(f"qc_{t}", [N, K], DT, kind="ExternalInput") for t in ("lo", "hi")}
    qrows_d = {t: nc.dram_tensor(f"qrows_{t}", [P, K], DT, kind="ExternalInput") for t in ("lo", "hi")}
    qrowsT_d = {t: nc.dram_tensor(f"qrowsT_{t}", [K, P], DT, kind="ExternalInput") for t in ("lo", "hi")}
    w99_d = {t: nc.dram_tensor(f"w99_{t}", [K, K], DT, kind="ExternalInput") for t in ("lo", "hi")}
    smid_d = nc.dram_tensor("smid", [128, 128], DT, kind="ExternalInput")
    sup_d = nc.dram_tensor("sup", [128, 128], DT, kind="ExternalInput")
    sdn_d = nc.dram_tensor("sdn", [128, 128], DT, kind="ExternalInput")
    ident_d = nc.dram_tensor("ident", [128, 128], DT, kind="ExternalInput")
    out_d = nc.dram_tensor("out", [N, P], DT, kind="ExternalOutput")

    import concourse.mybir as mb
    ACTF = mb.ActivationFunctionType
    LN025 = float(np.log(0.25))
    TS = ("lo", "hi")

    with tile.TileContext(nc) as tc:
        with tc.tile_pool(name="pers", bufs=1) as pers, \
             tc.tile_pool(name="rot", bufs=2) as rot, \
             tc.tile_pool(name="qct", bufs=1) as qctp, \
             tc.tile_pool(name="ps", bufs=1, space="PSUM") as ps, \
             tc.tile_pool(name="dram", bufs=2, space="DRAM") as dram:

            # ---- persistent SBUF arrays ----
            x0b = pers.tile([128, RC * PW], DT, tag="x0b")
            x1b = pers.tile([128, RC * P], DT, tag="x1b")
            qc_s = {t: pers.tile([128, RC * K], DT, tag=f"qc_{t}", name=f"qc_{t}") for t in TS}
            qrows_s = {t: pers.tile([128, 2 * K], DT, tag=f"qr_{t}", name=f"qr_{t}") for t in TS}
            qrowsT_s = {t: pers.tile([128, KC * P], DT, tag=f"qrt_{t}", name=f"qrt_{t}") for t in TS}
            w99_s = {t: pers.tile([128, KC * K], DT, tag=f"w99_{t}", name=f"w99_{t}") for t in TS}
            abuf = {t: pers.tile([128, 2 * K], DT, tag=f"ab_{t}", name=f"ab_{t}") for t in TS}
            ufix = {t: pers.tile([128, KC * K], DT, tag=f"uf_{t}", name=f"uf_{t}") for t in TS}
            utb = {t: pers.tile([128, KC * K], DT, tag=f"ut_{t}", name=f"ut_{t}") for t in TS}
            zbuf = {t: pers.tile([128, KC * P], DT, tag=f"zb_{t}", name=f"zb_{t}") for t in TS}
            outpart = pers.tile([128, RC * P], DT, tag="outpart")
            smid_s = pers.tile([128, 128], DT, tag="smid")
            sup_s = pers.tile([128, 128], DT, tag="sup")
            sdn_s = pers.tile([128, 128], DT, tag="sdn")
            ident_s = pers.tile([128, 128], DT, tag="ident")

            # ---- const APs for activation bias values ----
            for cv, cn in ((-0.5, "cneg05"), (LN025, "cln025")):
                ct = pers.tile([128, 1], DT, tag=cn, name=cn)
                nc.vector.memset(ct[:], cv)
                nc.const_aps.aps[(DT, float(cv))] = ct[:]

            # ---- constant loads ----
            nc.sync.dma_start(smid_s[:], smid_d[:, :])
            nc.sync.dma_start(sup_s[:], sup_d[:, :])
            nc.sync.dma_start(sdn_s[:], sdn_d[:, :])
            nc.sync.dma_start(ident_s[:], ident_d[:, :])
            for t in TS:
                for r in range(RC):
                    nc.sync.dma_start(qc_s[t][:, K * r:K * (r + 1)], qc_d[t][128 * r:128 * (r + 1), :])
                for kj in range(2):
                    nc.sync.dma_start(qrows_s[t][:, K * kj:K * (kj + 1)], qrows_d[t][128 * kj:128 * (kj + 1), :])
                for kb in range(KC):
                    nc.sync.dma_start(qrowsT_s[t][:, P * kb:P * (kb + 1)], qrowsT_d[t][128 * kb:128 * (kb + 1), :])
                    nc.sync.dma_start(w99_s[t][:, K * kb:K * (kb + 1)], w99_d[t][128 * kb:128 * (kb + 1), :])

            # ---- phase 0: x0' = 0.25*exp(-50((X-.5)^2+(Y-.5)^2)) per row chunk ----
            for r in range(RC):
                xt = rot.tile([128, PW], DT, tag="xt")
                yt = rot.tile([128, PW], DT, tag="yt")
                nc.sync.dma_start(xt[:], xin[128 * r:128 * (r + 1), :])
                nc.sync.dma_start(yt[:], yin[128 * r:128 * (r + 1), :])
                sq1 = rot.tile([128, PW], DT, tag="sq1")
                nc.scalar.activation(sq1[:], xt[:], ACTF.Square, bias=-0.5, scale=1.0)
                d2 = rot.tile([128, PW], DT, tag="d2")
                nc.vector.tensor_scalar_add(d2[:], yt[:], -0.5)
                nc.vector.tensor_mul(d2[:], d2[:], d2[:])
                nc.vector.tensor_add(sq1[:], sq1[:], d2[:])
                # 0.25*exp(v) == exp(v + ln(1/4))
                nc.scalar.activation(x0b[:, PW * r:PW * (r + 1)], sq1[:], ACTF.Exp,
                                     bias=LN025, scale=-50.0)

            # ---- phase 1: one explicit Jacobi step -> x1 (panel cols 1..256) ----
            for r in range(RC):
                vps = ps.tile([128, PW], DT, tag="pp", bufs=4, name="vps")
                first, last = True, False
                nc.tensor.matmul(vps[:], smid_s[:], x0b[:, PW * r:PW * (r + 1)],
                                 start=True, stop=(r == 0 and r == RC - 1))
                if r > 0:
                    nc.tensor.matmul(vps[:], sup_s[:], x0b[:, PW * (r - 1):PW * r],
                                     start=False, stop=(r == RC - 1))
                if r < RC - 1:
                    nc.tensor.matmul(vps[:], sdn_s[:], x0b[:, PW * (r + 1):PW * (r + 2)],
                                     start=False, stop=True)
                th = rot.tile([128, P], DT, tag="th")
                nc.vector.tensor_add(th[:], x0b[:, PW * r:PW * r + P],
                                     x0b[:, PW * r + 2:PW * r + 2 + P])
                nc.vector.tensor_add(x1b[:, P * r:P * (r + 1)], th[:], vps[:, 1:1 + P])

            # ---- forward mm1: A_t = x1^T @ Qc_t  (PSUM-accumulated over 16 row chunks) ----
            aps = {(t, jm): ps.tile([128, K], DT, tag="aacc", bufs=4, name=f"aps_{t}{jm}") for t in TS for jm in range(2)}
            for r in range(RC):
                for jm in range(2):
                    for t in TS:
                        nc.tensor.matmul(aps[(t, jm)][:],
                                         x1b[:, P * r + 128 * jm:P * r + 128 * (jm + 1)],
                                         qc_s[t][:, K * r:K * (r + 1)],
                                         start=(r == 0), stop=(r == RC - 1))
            for t in TS:
                for jm in range(2):
                    nc.vector.tensor_copy(abuf[t][:, K * jm:K * (jm + 1)], aps[(t, jm)][:])

            # ---- forward mm2: G_t = A_t^T @ Qrows_t -> DRAM for AllReduce ----
            gin = dram.tile([2 * K, K], DT, tag="gin")
            gout = dram.tile([2 * K, K], DT, tag="gout")
            for ti, t in enumerate(TS):
                for am in range(KC):
                    gps = ps.tile([128, K], DT, tag="pp", bufs=4, name="gps")
                    for kj in range(2):
                        nc.tensor.matmul(gps[:],
                                         abuf[t][:, K * kj + 128 * am:K * kj + 128 * (am + 1)],
                                         qrows_s[t][:, K * kj:K * (kj + 1)],
                                         start=(kj == 0), stop=(kj == 1))
                    gsb = rot.tile([128, K], DT, tag="sc", name="gsb")
                    nc.vector.tensor_copy(gsb[:], gps[:])
                    nc.sync.dma_start(gin[K * ti + 128 * am:K * ti + 128 * (am + 1), :], gsb[:])

            import concourse.mybir as mybir
            nc.gpsimd.collective_compute(
                "AllReduce", mybir.AluOpType.add,
                replica_groups=[list(range(NC))],
                ins=[gin.opt()], outs=[gout.opt()],
            )

            # ---- spectral filter + transpose ----
            for ti, t in enumerate(TS):
                for am in range(KC):
                    uraw = rot.tile([128, K], DT, tag="sc", name="uraw")
                    nc.sync.dma_start(uraw[:], gout[K * ti + 128 * am:K * ti + 128 * (am + 1), :])
                    nc.vector.tensor_mul(ufix[t][:, K * am:K * (am + 1)], uraw[:],
                                         w99_s[t][:, K * am:K * (am + 1)])
                for bm in range(KC):
                    pst = ps.tile([128, K], DT, tag="pp", bufs=4, name="pst")
                    for am in range(KC):
                        nc.tensor.transpose(pst[:, 128 * am:128 * (am + 1)],
                                            ufix[t][:, K * am + 128 * bm:K * am + 128 * (bm + 1)],
                                            ident_s[:])
                    nc.vector.tensor_copy(utb[t][:, K * bm:K * (bm + 1)], pst[:])

            # ---- backward B1: Z_t = Uhat_t @ QrowsT_t ----
            for t in TS:
                for am in range(KC):
                    zps = ps.tile([128, P], DT, tag="pp", bufs=4, name="zps")
                    for kb in range(KC):
                        nc.tensor.matmul(zps[:],
                                         utb[t][:, K * kb + 128 * am:K * kb + 128 * (am + 1)],
                                         qrowsT_s[t][:, P * kb:P * (kb + 1)],
                                         start=(kb == 0), stop=(kb == KC - 1))
                    nc.vector.tensor_copy(zbuf[t][:, P * am:P * (am + 1)], zps[:])

            # ---- backward B2: out = sum_t QcT_t^T @ Z_t, two passes over t ----
            for ti, t in enumerate(TS):
                qts = []
                for ka in range(KC):
                    qt_k = qctp.tile([128, N], DT, tag=f"qct{ka}")
                    for r in range(RC):
                        pst2 = ps.tile([128, 128], DT, tag="pp", bufs=4, name="pst2")
                        nc.tensor.transpose(pst2[:], qc_s[t][:, K * r + 128 * ka:K * r + 128 * (ka + 1)], ident_s[:])
                        nc.vector.tensor_copy(qt_k[:, 128 * r:128 * (r + 1)], pst2[:])
                    qts.append(qt_k)
                for r in range(RC):
                    ops = ps.tile([128, P], DT, tag="pp", bufs=4, name="ops")
                    for ka in range(KC):
                        nc.tensor.matmul(ops[:],
                                         qts[ka][:, 128 * r:128 * (r + 1)],
                                         zbuf[t][:, P * ka:P * (ka + 1)],
                                         start=(ka == 0), stop=(ka == KC - 1))
                    if ti == 0:
                        nc.vector.tensor_copy(outpart[:, P * r:P * (r + 1)], ops[:])
                    else:
                        osb = rot.tile([128, P], DT, tag="sc", name="osb")
                        nc.vector.tensor_add(osb[:], ops[:], outpart[:, P * r:P * (r + 1)])
                        nc.sync.dma_start(out_d[128 * r:128 * (r + 1), :], osb[:])

    nc.compile()
    _NC_CACHE["nc"] = nc
    return nc


def _run(X, Y, trace=False):
    _install_ntff_hook()
    from concourse.bass_utils import run_bass_kernel_spmd

    X = np.asarray(X, dtype=np.float32)
    Y = np.asarray(Y, dtype=np.float32)
    consts = _host_constants()
    Xp = np.zeros((N, N + 2), np.float32); Xp[:, 1:-1] = X
    Yp = np.zeros((N, N + 2), np.float32); Yp[:, 1:-1] = Y

    in_maps = []
    for c in range(NC):
        m = {"X": Xp[:, P * c:P * c + PW], "Y": Yp[:, P * c:P * c + PW]}
        for t in ("lo", "hi"):
            m[f"qc_{t}"] = consts[f"qc_{t}"]
            m[f"qrows_{t}"] = consts[f"qc_{t}"][P * c:P * (c + 1), :]
            m[f"qrowsT_{t}"] = np.ascontiguousarray(consts[f"qc_{t}"][P * c:P * (c + 1), :].T)
            m[f"w99_{t}"] = consts[f"w99_{t}"]
        for k in ("smid", "sup", "sdn", "ident"):
            m[k] = consts[k]
        in_maps.append(m)

    nc = _build()
    r = run_bass_kernel_spmd(nc, in_maps, core_ids=list(range(NC)), trace=trace)
    panels = [r.results[c]["out"] for c in range(NC)]
    full = np.concatenate(panels, axis=1).astype(np.float32)
    return full[None, None], r


def kernel(X, Y):
    out, _ = _run(X, Y, trace=False)
    return out



# revision 3
# speedup vs baseline: 1.5210x; 1.5210x over previous
"""Jacobi 100-step solver on 8 trn2 cores via truncated DST-spectral transform.

x_{t+1} = mask * (0.25 * 4-neighbor-sum) is linear and diagonalizes in the DST
basis Q: after one explicit step, x100 = Q (s^99 . (Q x1 Q)) Q with
s = 0.5(cos a + cos b). |s|^99 is negligible outside the lowest-K and highest-K
mode corners (K=256 -> rel err ~6.5e-3 incl fp16 noise, gate 2e-2). Everything
on-device runs in fp16 (1-pass PE matmuls, fp32 PSUM accumulation); the
spectral AllReduce payload is one [512,256] fp16 block. Sharding: 256-column
panels per core.
"""

import sys
import types
import numpy as np

N = 2048
NC = 8
P = N // NC          # 256 panel columns per core
K = 256              # spectral corner size per corner
K2 = 2 * K           # lo|hi concatenated
PW = P + 2           # panel width with 1-col halos
RC = N // 128        # 16 row chunks


def _install_ntff_hook():
    if "antenv.axon_hooks" in sys.modules:
        return
    mod = types.ModuleType("antenv.axon_hooks")
    mod._hook = None
    mod.set_axon_ntff_profile_hook = lambda h: setattr(mod, "_hook", h)
    mod.get_axon_ntff_profile_hook = lambda: mod._hook
    sys.modules["antenv.axon_hooks"] = mod
    try:
        import antenv
        antenv.axon_hooks = mod
        from trn_agent_boot.trn_boot import _ntff_profile_via_ctypes
        h = _ntff_profile_via_ctypes("/opt/axon/libaxon_pjrt.so")
        if h is not None:
            mod.set_axon_ntff_profile_hook(h)
    except Exception:
        pass


def _host_constants():
    i = np.arange(N, dtype=np.float64)
    qcs, qcTs, w99s = [], [], []
    for lo in (True, False):
        m = np.arange(1, K + 1, dtype=np.float64) if lo else np.arange(N - 1 - K, N - 1, dtype=np.float64)
        red = np.outer(i, m) % (2 * (N - 1))
        Qc = np.sqrt(2.0 / (N - 1)) * np.sin(np.pi * red / (N - 1))   # [2048, K]
        lam = 0.5 * np.cos(np.pi * m / (N - 1))
        W99 = (lam[:, None] + lam[None, :]) ** 99                     # [K, K]
        qcs.append(Qc.astype(np.float16))
        qcTs.append(np.ascontiguousarray(Qc.T).astype(np.float16))
        w99s.append(W99.astype(np.float16))
    consts = {
        "qcb": np.ascontiguousarray(np.concatenate(qcs, axis=1)),     # [2048, 512]
        "qcTb": np.ascontiguousarray(np.concatenate(qcTs, axis=0)),   # [512, 2048]
        "w99b": np.ascontiguousarray(np.concatenate(w99s, axis=0)),   # [512, 256]
    }
    smid = np.zeros((128, 128), np.float16)
    for d in range(127):
        smid[d, d + 1] = 1.0
        smid[d + 1, d] = 1.0
    sup = np.zeros((128, 128), np.float16); sup[127, 0] = 1.0
    sdn = np.zeros((128, 128), np.float16); sdn[0, 127] = 1.0
    consts["smid"], consts["sup"], consts["sdn"] = smid, sup, sdn
    consts["ident"] = np.eye(128, dtype=np.float16)
    return consts


_NC_CACHE = {}


def _build():
    if "nc" in _NC_CACHE:
        return _NC_CACHE["nc"]
    import concourse.bacc as bacc
    import concourse.tile as tile
    import concourse.mybir as mybir

    F16 = mybir.dt.float16
    F32 = mybir.dt.float32
    nc = bacc.Bacc("TRN2", target_bir_lowering=False, debug=False, num_devices=NC)

    xin = nc.dram_tensor("X", [N, PW], F16, kind="ExternalInput")
    yin = nc.dram_tensor("Y", [N, PW], F16, kind="ExternalInput")
    qcb_d = nc.dram_tensor("qcb", [N, K2], F16, kind="ExternalInput")
    qcTb_d = nc.dram_tensor("qcTb", [K2, N], F16, kind="ExternalInput")
    qrowsb_d = nc.dram_tensor("qrowsb", [P, K2], F16, kind="ExternalInput")
    qrowsTb_d = nc.dram_tensor("qrowsTb", [K2, P], F16, kind="ExternalInput")
    w99b_d = nc.dram_tensor("w99b", [K2, K], F16, kind="ExternalInput")
    smid_d = nc.dram_tensor("smid", [128, 128], F16, kind="ExternalInput")
    sup_d = nc.dram_tensor("sup", [128, 128], F16, kind="ExternalInput")
    sdn_d = nc.dram_tensor("sdn", [128, 128], F16, kind="ExternalInput")
    ident_d = nc.dram_tensor("ident", [128, 128], F16, kind="ExternalInput")
    out_d = nc.dram_tensor("out", [N, P], F16, kind="ExternalOutput")

    ACTF = mybir.ActivationFunctionType
    LN025 = float(np.log(0.25))

    with tile.TileContext(nc) as tc:
        with tc.tile_pool(name="pers", bufs=1) as pers, \
             tc.tile_pool(name="rot", bufs=3) as rot, \
             tc.tile_pool(name="ps", bufs=1, space="PSUM") as ps, \
             tc.tile_pool(name="dram", bufs=2, space="DRAM") as dram:

            # ---- persistent SBUF ----
            x0b = pers.tile([128, RC * PW], F16, tag="x0b")
            hsum = pers.tile([128, RC * P], F16, tag="hsum")
            x1b = pers.tile([128, RC * P], F16, tag="x1b")
            qcb_s = pers.tile([128, RC * K2], F16, tag="qcb")
            qcTb_s = pers.tile([128, 4 * N], F16, tag="qcTb")
            qrowsb_s = pers.tile([128, 2 * K2], F16, tag="qrb")
            qrowsTb_s = pers.tile([128, 4 * P], F16, tag="qrtb")
            w99_s = pers.tile([128, 4 * K], F16, tag="w99")
            abuf = pers.tile([128, 2 * K2], F16, tag="abuf")
            gsb = pers.tile([128, 4 * K], F16, tag="gsb")
            ufix = pers.tile([128, 4 * K], F16, tag="ufix")
            utb = pers.tile([128, 4 * K], F16, tag="utb")
            zbuf = pers.tile([128, 4 * P], F16, tag="zbuf")
            smid_s = pers.tile([128, 128], F16, tag="smid")
            sup_s = pers.tile([128, 128], F16, tag="sup")
            sdn_s = pers.tile([128, 128], F16, tag="sdn")
            ident_s = pers.tile([128, 128], F16, tag="ident")

            # const APs for activation bias values
            for cv, cn in ((-0.5, "cneg05"), (LN025, "cln025")):
                ct = pers.tile([128, 1], F32, tag=cn, name=cn)
                nc.vector.memset(ct[:], cv)
                nc.const_aps.aps[(F32, float(cv))] = ct[:]

            # ---- stencil consts ----
            nc.sync.dma_start(smid_s[:], smid_d[:, :])
            nc.sync.dma_start(sup_s[:], sup_d[:, :])
            nc.sync.dma_start(sdn_s[:], sdn_d[:, :])
            nc.sync.dma_start(ident_s[:], ident_d[:, :])

            # ---- phase 0: x0 = 0.25*exp(-50((X-.5)^2+(Y-.5)^2)), pipelined with DMA ----
            for r in range(RC):
                xt = rot.tile([128, PW], F16, tag="xt")
                yt = rot.tile([128, PW], F16, tag="yt")
                nc.sync.dma_start(xt[:], xin[128 * r:128 * (r + 1), :])
                nc.sync.dma_start(yt[:], yin[128 * r:128 * (r + 1), :])
                # prefetch qc row-chunk on the side
                nc.sync.dma_start(qcb_s[:, K2 * r:K2 * (r + 1)], qcb_d[128 * r:128 * (r + 1), :])

                sqx = rot.tile([128, PW], F16, tag="sqx")
                if r % 4 == 0:
                    nc.scalar.activation(sqx[:], xt[:], ACTF.Square, bias=-0.5, scale=1.0)
                elif r % 4 == 2:
                    tx = rot.tile([128, PW], F16, tag="tx")
                    nc.gpsimd.tensor_scalar_add(tx[:], xt[:], -0.5)
                    nc.gpsimd.tensor_mul(sqx[:], tx[:], tx[:])
                else:
                    tx = rot.tile([128, PW], F16, tag="tx")
                    nc.vector.tensor_scalar_add(tx[:], xt[:], -0.5)
                    nc.vector.tensor_mul(sqx[:], tx[:], tx[:])
                ty = rot.tile([128, PW], F16, tag="ty")
                sqy = rot.tile([128, PW], F16, tag="sqy")
                nc.gpsimd.tensor_scalar_add(ty[:], yt[:], -0.5)
                nc.gpsimd.tensor_mul(sqy[:], ty[:], ty[:])
                d2 = rot.tile([128, PW], F16, tag="d2")
                nc.vector.tensor_add(d2[:], sqx[:], sqy[:])
                # 0.25*exp(-50 d) == exp(-50 d + ln(1/4))
                nc.scalar.activation(x0b[:, PW * r:PW * (r + 1)], d2[:], ACTF.Exp,
                                     bias=LN025, scale=-50.0)

            # ---- remaining const loads (needed later; overlap with compute) ----
            for kj in range(2):
                nc.sync.dma_start(qrowsb_s[:, K2 * kj:K2 * (kj + 1)], qrowsb_d[128 * kj:128 * (kj + 1), :])
            for b in range(4):
                nc.sync.dma_start(w99_s[:, K * b:K * (b + 1)], w99b_d[128 * b:128 * (b + 1), :])
                nc.sync.dma_start(qrowsTb_s[:, P * b:P * (b + 1)], qrowsTb_d[128 * b:128 * (b + 1), :])
                for seg in range(4):
                    nc.sync.dma_start(qcTb_s[:, N * b + 512 * seg:N * b + 512 * (seg + 1)],
                                      qcTb_d[128 * b:128 * (b + 1), 512 * seg:512 * (seg + 1)])

            # ---- phase H + phase 1: one explicit Jacobi step -> x1 (fp16) ----
            for r in range(RC):
                nc.gpsimd.tensor_add(hsum[:, P * r:P * (r + 1)],
                                     x0b[:, PW * r:PW * r + P],
                                     x0b[:, PW * r + 2:PW * r + 2 + P])
            for r in range(RC):
                vps = ps.tile([128, P], F32, tag="pp", bufs=4, name="vps")
                last_is_sdn = r < RC - 1
                nc.tensor.matmul(vps[:], smid_s[:], x0b[:, PW * r + 1:PW * r + 1 + P],
                                 start=True, stop=False)
                if r > 0:
                    nc.tensor.matmul(vps[:], sup_s[:], x0b[:, PW * (r - 1) + 1:PW * (r - 1) + 1 + P],
                                     start=False, stop=not last_is_sdn)
                if last_is_sdn:
                    nc.tensor.matmul(vps[:], sdn_s[:], x0b[:, PW * (r + 1) + 1:PW * (r + 1) + 1 + P],
                                     start=False, stop=True)
                nc.vector.tensor_add(x1b[:, P * r:P * (r + 1)], vps[:], hsum[:, P * r:P * (r + 1)])

            # ---- mm1: A = x1^T @ [Qc_lo|Qc_hi], PSUM-accumulated over 16 row chunks ----
            aps = [ps.tile([128, K2], F32, tag="aacc", bufs=2, name=f"aps{jm}") for jm in range(2)]
            for r in range(RC):
                for jm in range(2):
                    nc.tensor.matmul(aps[jm][:],
                                     x1b[:, P * r + 128 * jm:P * r + 128 * (jm + 1)],
                                     qcb_s[:, K2 * r:K2 * (r + 1)],
                                     start=(r == 0), stop=(r == RC - 1))
            for jm in range(2):
                nc.vector.tensor_copy(abuf[:, K2 * jm:K2 * (jm + 1)], aps[jm][:])

            # ---- mm2: G_t = A_t^T @ Qrows_t -> DRAM for AllReduce (fp16 payload) ----
            gin = dram.tile([K2, K], F16, tag="gin")
            gout = dram.tile([K2, K], F16, tag="gout")
            for ti in range(2):
                for am in range(2):
                    gps = ps.tile([128, K], F32, tag="pp", bufs=4, name="gps")
                    for kj in range(2):
                        nc.tensor.matmul(gps[:],
                                         abuf[:, K2 * kj + K * ti + 128 * am:K2 * kj + K * ti + 128 * (am + 1)],
                                         qrowsb_s[:, K2 * kj + K * ti:K2 * kj + K * (ti + 1)],
                                         start=(kj == 0), stop=(kj == 1))
                    b = 2 * ti + am
                    nc.scalar.copy(gsb[:, K * b:K * (b + 1)], gps[:])
                    nc.sync.dma_start(gin[128 * b:128 * (b + 1), :], gsb[:, K * b:K * (b + 1)])

            nc.gpsimd.collective_compute(
                "AllReduce", mybir.AluOpType.add,
                replica_groups=[list(range(NC))],
                ins=[gin.opt()], outs=[gout.opt()],
            )

            # ---- spectral filter (fp16 mul with W99) + transpose ----
            for b in range(4):
                uraw = rot.tile([128, K], F16, tag="uraw")
                nc.sync.dma_start(uraw[:], gout[128 * b:128 * (b + 1), :])
                nc.vector.tensor_mul(ufix[:, K * b:K * (b + 1)], uraw[:],
                                     w99_s[:, K * b:K * (b + 1)])
            for ti in range(2):
                for bm in range(2):
                    for am in range(2):
                        pst = ps.tile([128, 128], F16, tag="pt", bufs=2, name="pst")
                        nc.tensor.transpose(pst[:],
                                            ufix[:, K * (2 * ti + am) + 128 * bm:K * (2 * ti + am) + 128 * (bm + 1)],
                                            ident_s[:])
                        nc.scalar.copy(utb[:, K * (2 * ti + bm) + 128 * am:K * (2 * ti + bm) + 128 * (am + 1)], pst[:])

            # ---- B1: Z_t = Uhat_t @ QrowsT_t ----
            for ti in range(2):
                for am in range(2):
                    zps = ps.tile([128, P], F32, tag="pp", bufs=4, name="zps")
                    for kb in range(2):
                        nc.tensor.matmul(zps[:],
                                         utb[:, K * (2 * ti + kb) + 128 * am:K * (2 * ti + kb) + 128 * (am + 1)],
                                         qrowsTb_s[:, P * (2 * ti + kb):P * (2 * ti + kb + 1)],
                                         start=(kb == 0), stop=(kb == 1))
                    nc.vector.tensor_copy(zbuf[:, P * (2 * ti + am):P * (2 * ti + am + 1)], zps[:])

            # ---- B2: out_r = sum_{t,ka} QcT_{t,ka,r}^T @ Z_{t,ka} ----
            for r in range(RC):
                ops = ps.tile([128, P], F32, tag="pp", bufs=4, name="ops")
                for ti in range(2):
                    for ka in range(2):
                        b = 2 * ti + ka
                        nc.tensor.matmul(ops[:],
                                         qcTb_s[:, N * b + 128 * r:N * b + 128 * (r + 1)],
                                         zbuf[:, P * b:P * (b + 1)],
                                         start=(b == 0), stop=(b == 3))
                osb = rot.tile([128, P], F16, tag="osb")
                if r % 2 == 0:
                    nc.vector.tensor_copy(osb[:], ops[:])
                else:
                    nc.scalar.copy(osb[:], ops[:])
                nc.sync.dma_start(out_d[128 * r:128 * (r + 1), :], osb[:])

    nc.compile()
    _NC_CACHE["nc"] = nc
    return nc


def _run(X, Y, trace=False):
    _install_ntff_hook()
    from concourse.bass_utils import run_bass_kernel_spmd

    consts = _host_constants()
    Xp = np.zeros((N, N + 2), np.float16); Xp[:, 1:-1] = np.asarray(X, np.float32).astype(np.float16)
    Yp = np.zeros((N, N + 2), np.float16); Yp[:, 1:-1] = np.asarray(Y, np.float32).astype(np.float16)

    in_maps = []
    for c in range(NC):
        m = {"X": np.ascontiguousarray(Xp[:, P * c:P * c + PW]),
             "Y": np.ascontiguousarray(Yp[:, P * c:P * c + PW]),
             "qcb": consts["qcb"],
             "qcTb": consts["qcTb"],
             "w99b": consts["w99b"],
             "qrowsb": np.ascontiguousarray(consts["qcb"][P * c:P * (c + 1), :]),
             "qrowsTb": np.ascontiguousarray(consts["qcTb"][:, P * c:P * (c + 1)]),
             "smid": consts["smid"], "sup": consts["sup"], "sdn": consts["sdn"],
             "ident": consts["ident"]}
        in_maps.append(m)

    nc = _build()
    r = run_bass_kernel_spmd(nc, in_maps, core_ids=list(range(NC)), trace=trace)
    panels = [r.results[c]["out"] for c in range(NC)]
    full = np.concatenate(panels, axis=1).astype(np.float32)
    return full[None, None], r


def kernel(X, Y):
    out, _ = _run(X, Y, trace=False)
    return out


# revision 7
# speedup vs baseline: 2.2338x; 1.4686x over previous
"""Jacobi 100-step solver on 8 trn2 cores via truncated DST-spectral transform.

x_{t+1} = mask * (0.25 * 4-neighbor-sum) is linear and diagonalizes in the DST
basis Q: after one explicit step, x100 = Q (s^99 . (Q x1 Q)) Q with
s = 0.5(cos a + cos b). |s|^99 is negligible outside the lowest-K and highest-K
mode corners (K=256 -> rel err ~6.5e-3 incl fp16 noise, gate 2e-2). Everything
on-device runs in fp16 (1-pass PE matmuls, fp32 PSUM accumulation); the
spectral AllReduce payload is one [512,256] fp16 block. Sharding: 256-column
panels per core.
"""

import sys
import types
import numpy as np

N = 2048
NC = 8
P = N // NC          # 256 panel columns per core
K = 256              # spectral corner size per corner
K2 = 2 * K           # lo|hi concatenated
PW = P + 2           # panel width with 1-col halos
RC = N // 128        # 16 row chunks


def _install_ntff_hook():
    if "antenv.axon_hooks" in sys.modules:
        return
    mod = types.ModuleType("antenv.axon_hooks")
    mod._hook = None
    mod.set_axon_ntff_profile_hook = lambda h: setattr(mod, "_hook", h)
    mod.get_axon_ntff_profile_hook = lambda: mod._hook
    sys.modules["antenv.axon_hooks"] = mod
    try:
        import antenv
        antenv.axon_hooks = mod
        from trn_agent_boot.trn_boot import _ntff_profile_via_ctypes
        h = _ntff_profile_via_ctypes("/opt/axon/libaxon_pjrt.so")
        if h is not None:
            mod.set_axon_ntff_profile_hook(h)
    except Exception:
        pass


def _host_constants():
    i = np.arange(N, dtype=np.float64)
    qcs, qcTs, w99s = [], [], []
    for lo in (True, False):
        m = np.arange(1, K + 1, dtype=np.float64) if lo else np.arange(N - 1 - K, N - 1, dtype=np.float64)
        red = np.outer(i, m) % (2 * (N - 1))
        Qc = np.sqrt(2.0 / (N - 1)) * np.sin(np.pi * red / (N - 1))   # [2048, K]
        lam = 0.5 * np.cos(np.pi * m / (N - 1))
        W99 = (lam[:, None] + lam[None, :]) ** 99                     # [K, K]
        qcs.append(Qc.astype(np.float16))
        qcTs.append(np.ascontiguousarray(Qc.T).astype(np.float16))
        w99s.append(W99.astype(np.float16))
    consts = {
        "qcb": np.ascontiguousarray(np.concatenate(qcs, axis=1)),     # [2048, 512]
        "qcTb": np.ascontiguousarray(np.concatenate(qcTs, axis=0)),   # [512, 2048]
        "w99b": np.ascontiguousarray(np.concatenate(w99s, axis=0)),   # [512, 256]
    }
    smid = np.zeros((128, 128), np.float16)
    for d in range(127):
        smid[d, d + 1] = 1.0
        smid[d + 1, d] = 1.0
    sup = np.zeros((128, 128), np.float16); sup[127, 0] = 1.0
    sdn = np.zeros((128, 128), np.float16); sdn[0, 127] = 1.0
    consts["smid"], consts["sup"], consts["sdn"] = smid, sup, sdn
    consts["ident"] = np.eye(128, dtype=np.float16)
    return consts


_NC_CACHE = {}


def _build():
    if "nc" in _NC_CACHE:
        return _NC_CACHE["nc"]
    import concourse.bacc as bacc
    import concourse.tile as tile
    import concourse.mybir as mybir

    F16 = mybir.dt.float16
    F32 = mybir.dt.float32
    nc = bacc.Bacc("TRN2", target_bir_lowering=False, debug=False, num_devices=NC)

    xin = nc.dram_tensor("X", [N, PW], F32, kind="ExternalInput")
    yin = nc.dram_tensor("Y", [N, PW], F32, kind="ExternalInput")
    qcb_d = nc.dram_tensor("qcb", [N, K2], F16, kind="ExternalInput")
    qcTb_d = nc.dram_tensor("qcTb", [K2, N], F16, kind="ExternalInput")
    qrowsb_d = nc.dram_tensor("qrowsb", [P, K2], F16, kind="ExternalInput")
    qrowsTb_d = nc.dram_tensor("qrowsTb", [K2, P], F16, kind="ExternalInput")
    w99b_d = nc.dram_tensor("w99b", [K2, K], F16, kind="ExternalInput")
    smid_d = nc.dram_tensor("smid", [128, 128], F16, kind="ExternalInput")
    sup_d = nc.dram_tensor("sup", [128, 128], F16, kind="ExternalInput")
    sdn_d = nc.dram_tensor("sdn", [128, 128], F16, kind="ExternalInput")
    ident_d = nc.dram_tensor("ident", [128, 128], F16, kind="ExternalInput")
    out_d = nc.dram_tensor("out", [N, P], F16, kind="ExternalOutput")

    ACTF = mybir.ActivationFunctionType
    LN025 = float(np.log(0.25))

    with tile.TileContext(nc) as tc:
        with tc.tile_pool(name="pers", bufs=1) as pers, \
             tc.tile_pool(name="rot", bufs=3) as rot, \
             tc.tile_pool(name="ps", bufs=1, space="PSUM") as ps, \
             tc.tile_pool(name="dram", bufs=2, space="DRAM") as dram:

            # ---- persistent SBUF ----
            x0b = pers.tile([128, RC * PW], F16, tag="x0b")
            hsum = pers.tile([128, RC * P], F16, tag="hsum")
            x1b = pers.tile([128, RC * P], F16, tag="x1b")
            qcb_s = pers.tile([128, RC * K2], F16, tag="qcb")
            qcTb_s = pers.tile([128, 4 * N], F16, tag="qcTb")
            qrowsb_s = pers.tile([128, 2 * K2], F16, tag="qrb")
            qrowsTb_s = pers.tile([128, 4 * P], F16, tag="qrtb")
            w99_s = pers.tile([128, 4 * K], F16, tag="w99")
            abuf = pers.tile([128, 2 * K2], F16, tag="abuf")
            gsb = pers.tile([128, 4 * K], F16, tag="gsb")
            ufix = pers.tile([128, 4 * K], F16, tag="ufix")
            utb = pers.tile([128, 4 * K], F16, tag="utb")
            zbuf = pers.tile([128, 4 * P], F16, tag="zbuf")
            smid_s = pers.tile([128, 128], F16, tag="smid")
            sup_s = pers.tile([128, 128], F16, tag="sup")
            sdn_s = pers.tile([128, 128], F16, tag="sdn")
            ident_s = pers.tile([128, 128], F16, tag="ident")

            # const APs for activation bias values
            for cv, cn in ((-0.5, "cneg05"), (LN025, "cln025")):
                ct = pers.tile([128, 1], F32, tag=cn, name=cn)
                nc.vector.memset(ct[:], cv)
                nc.const_aps.aps[(F32, float(cv))] = ct[:]

            # ---- stencil consts ----
            nc.sync.dma_start(smid_s[:], smid_d[:, :])
            nc.sync.dma_start(sup_s[:], sup_d[:, :])
            nc.sync.dma_start(sdn_s[:], sdn_d[:, :])
            nc.sync.dma_start(ident_s[:], ident_d[:, :])

            # ---- phase 0: x0 = 0.25*exp(-50((X-.5)^2+(Y-.5)^2)), pipelined with DMA ----
            # fp32 elementwise (fp16 two-operand DVE ops hit a slow microcode path);
            # only the final Exp writes fp16.
            for r in range(RC):
                xt = rot.tile([128, PW], F32, tag="xt")
                yt = rot.tile([128, PW], F32, tag="yt")
                nc.sync.dma_start(xt[:], xin[128 * r:128 * (r + 1), :])
                nc.sync.dma_start(yt[:], yin[128 * r:128 * (r + 1), :])
                # prefetch qc row-chunk on the side
                nc.sync.dma_start(qcb_s[:, K2 * r:K2 * (r + 1)], qcb_d[128 * r:128 * (r + 1), :])

                sqx = rot.tile([128, PW], F32, tag="sqx")
                nc.scalar.activation(sqx[:], xt[:], ACTF.Square, bias=-0.5, scale=1.0)
                ty = rot.tile([128, PW], F32, tag="ty")
                sqy = rot.tile([128, PW], F32, tag="sqy")
                nc.vector.tensor_scalar_add(ty[:], yt[:], -0.5)
                nc.vector.tensor_mul(sqy[:], ty[:], ty[:])
                d2 = rot.tile([128, PW], F32, tag="d2")
                if r % 2 == 0:
                    nc.gpsimd.tensor_add(d2[:], sqx[:], sqy[:])
                else:
                    nc.vector.tensor_add(d2[:], sqx[:], sqy[:])
                # 0.25*exp(-50 d) == exp(-50 d + ln(1/4))
                nc.scalar.activation(x0b[:, PW * r:PW * (r + 1)], d2[:], ACTF.Exp,
                                     bias=LN025, scale=-50.0)

            # ---- remaining const loads (needed later; overlap with compute) ----
            for kj in range(2):
                nc.sync.dma_start(qrowsb_s[:, K2 * kj:K2 * (kj + 1)], qrowsb_d[128 * kj:128 * (kj + 1), :])
            for b in range(4):
                nc.sync.dma_start(w99_s[:, K * b:K * (b + 1)], w99b_d[128 * b:128 * (b + 1), :])
                nc.sync.dma_start(qrowsTb_s[:, P * b:P * (b + 1)], qrowsTb_d[128 * b:128 * (b + 1), :])
                for seg in range(4):
                    nc.sync.dma_start(qcTb_s[:, N * b + 512 * seg:N * b + 512 * (seg + 1)],
                                      qcTb_d[128 * b:128 * (b + 1), 512 * seg:512 * (seg + 1)])

            # ---- phase H + phase 1: one explicit Jacobi step -> x1 (fp16) ----
            for r in range(RC):
                nc.gpsimd.tensor_add(hsum[:, P * r:P * (r + 1)],
                                     x0b[:, PW * r:PW * r + P],
                                     x0b[:, PW * r + 2:PW * r + 2 + P])
            for r in range(RC):
                vps = ps.tile([128, P], F32, tag="pp", bufs=4, name="vps")
                last_is_sdn = r < RC - 1
                nc.tensor.matmul(vps[:], smid_s[:], x0b[:, PW * r + 1:PW * r + 1 + P],
                                 start=True, stop=False)
                if r > 0:
                    nc.tensor.matmul(vps[:], sup_s[:], x0b[:, PW * (r - 1) + 1:PW * (r - 1) + 1 + P],
                                     start=False, stop=not last_is_sdn)
                if last_is_sdn:
                    nc.tensor.matmul(vps[:], sdn_s[:], x0b[:, PW * (r + 1) + 1:PW * (r + 1) + 1 + P],
                                     start=False, stop=True)
                nc.vector.tensor_add(x1b[:, P * r:P * (r + 1)], vps[:], hsum[:, P * r:P * (r + 1)])

            # ---- mm1: A = x1^T @ [Qc_lo|Qc_hi], PSUM-accumulated over 16 row chunks ----
            aps = [ps.tile([128, K2], F32, tag="aacc", bufs=2, name=f"aps{jm}") for jm in range(2)]
            for r in range(RC):
                for jm in range(2):
                    nc.tensor.matmul(aps[jm][:],
                                     x1b[:, P * r + 128 * jm:P * r + 128 * (jm + 1)],
                                     qcb_s[:, K2 * r:K2 * (r + 1)],
                                     start=(r == 0), stop=(r == RC - 1))
            for jm in range(2):
                nc.vector.tensor_copy(abuf[:, K2 * jm:K2 * (jm + 1)], aps[jm][:])

            # ---- mm2: G_t = A_t^T @ Qrows_t -> DRAM for AllReduce (fp16 payload) ----
            gin = dram.tile([K2, K], F16, tag="gin")
            gout = dram.tile([K2, K], F16, tag="gout", addr_space="Shared")
            for ti in range(2):
                for am in range(2):
                    gps = ps.tile([128, K], F32, tag="pp", bufs=4, name="gps")
                    for kj in range(2):
                        nc.tensor.matmul(gps[:],
                                         abuf[:, K2 * kj + K * ti + 128 * am:K2 * kj + K * ti + 128 * (am + 1)],
                                         qrowsb_s[:, K2 * kj + K * ti:K2 * kj + K * (ti + 1)],
                                         start=(kj == 0), stop=(kj == 1))
                    b = 2 * ti + am
                    nc.scalar.copy(gsb[:, K * b:K * (b + 1)], gps[:])
                    nc.sync.dma_start(gin[128 * b:128 * (b + 1), :], gsb[:, K * b:K * (b + 1)])

            nc.gpsimd.collective_compute(
                "AllReduce", mybir.AluOpType.add,
                replica_groups=[list(range(NC))],
                ins=[gin.opt()], outs=[gout.opt()],
            )

            # ---- spectral filter (fp16 mul with W99) + transpose ----
            for b in range(4):
                uraw = rot.tile([128, K], F16, tag="uraw")
                nc.sync.dma_start(uraw[:], gout[128 * b:128 * (b + 1), :])
                nc.vector.tensor_mul(ufix[:, K * b:K * (b + 1)], uraw[:],
                                     w99_s[:, K * b:K * (b + 1)])
            for ti in range(2):
                for bm in range(2):
                    for am in range(2):
                        pst = ps.tile([128, 128], F16, tag="pt", bufs=2, name="pst")
                        nc.tensor.transpose(pst[:],
                                            ufix[:, K * (2 * ti + am) + 128 * bm:K * (2 * ti + am) + 128 * (bm + 1)],
                                            ident_s[:])
                        nc.scalar.copy(utb[:, K * (2 * ti + bm) + 128 * am:K * (2 * ti + bm) + 128 * (am + 1)], pst[:])

            # ---- B1: Z_t = Uhat_t @ QrowsT_t ----
            for ti in range(2):
                for am in range(2):
                    zps = ps.tile([128, P], F32, tag="pp", bufs=4, name="zps")
                    for kb in range(2):
                        nc.tensor.matmul(zps[:],
                                         utb[:, K * (2 * ti + kb) + 128 * am:K * (2 * ti + kb) + 128 * (am + 1)],
                                         qrowsTb_s[:, P * (2 * ti + kb):P * (2 * ti + kb + 1)],
                                         start=(kb == 0), stop=(kb == 1))
                    nc.vector.tensor_copy(zbuf[:, P * (2 * ti + am):P * (2 * ti + am + 1)], zps[:])

            # ---- B2: out_r = sum_{t,ka} QcT_{t,ka,r}^T @ Z_{t,ka} ----
            for r in range(RC):
                ops = ps.tile([128, P], F32, tag="pp", bufs=4, name="ops")
                for ti in range(2):
                    for ka in range(2):
                        b = 2 * ti + ka
                        nc.tensor.matmul(ops[:],
                                         qcTb_s[:, N * b + 128 * r:N * b + 128 * (r + 1)],
                                         zbuf[:, P * b:P * (b + 1)],
                                         start=(b == 0), stop=(b == 3))
                osb = rot.tile([128, P], F16, tag="osb")
                if r % 2 == 0:
                    nc.vector.tensor_copy(osb[:], ops[:])
                else:
                    nc.scalar.copy(osb[:], ops[:])
                nc.sync.dma_start(out_d[128 * r:128 * (r + 1), :], osb[:])

    nc.compile()
    _NC_CACHE["nc"] = nc
    return nc


def _run(X, Y, trace=False):
    _install_ntff_hook()
    from concourse.bass_utils import run_bass_kernel_spmd

    consts = _host_constants()
    Xp = np.zeros((N, N + 2), np.float32); Xp[:, 1:-1] = np.asarray(X, np.float32)
    Yp = np.zeros((N, N + 2), np.float32); Yp[:, 1:-1] = np.asarray(Y, np.float32)

    in_maps = []
    for c in range(NC):
        m = {"X": np.ascontiguousarray(Xp[:, P * c:P * c + PW]),
             "Y": np.ascontiguousarray(Yp[:, P * c:P * c + PW]),
             "qcb": consts["qcb"],
             "qcTb": consts["qcTb"],
             "w99b": consts["w99b"],
             "qrowsb": np.ascontiguousarray(consts["qcb"][P * c:P * (c + 1), :]),
             "qrowsTb": np.ascontiguousarray(consts["qcTb"][:, P * c:P * (c + 1)]),
             "smid": consts["smid"], "sup": consts["sup"], "sdn": consts["sdn"],
             "ident": consts["ident"]}
        in_maps.append(m)

    nc = _build()
    r = run_bass_kernel_spmd(nc, in_maps, core_ids=list(range(NC)), trace=trace)
    panels = [r.results[c]["out"] for c in range(NC)]
    full = np.concatenate(panels, axis=1).astype(np.float32)
    return full[None, None], r


def kernel(X, Y):
    out, _ = _run(X, Y, trace=False)
    return out
